# revision 77
# baseline (speedup 1.0000x reference)
"""Trainium2 Bass kernel for a dense decoder layer (GQA attention + gated MLP).

Distribution (8 NeuronCores): DP=2 over batch x TP=4 megatron split with
sequence-parallel norms (Megatron-SP style).
  - cores 0-3: batch 0, model ranks 0-3; cores 4-7: batch 1.
  - wq/wk/wv column-sharded by heads (8 q / 2 kv heads per core), wo row-sharded;
    w_gate/w_val column-sharded, w_out row-sharded (FFN padded 10928->11264).
  - Collectives per 4-core group (all bf16): ReduceScatter(h_attn partial) over
    tokens -> post-attn norm on 256 tokens/rank -> AllGather(hn2) -> MLP ->
    ReduceScatter(h_dense partial) -> final norm on 256 tokens/rank. Each core
    emits its 256-token output slice; the host reassembles.

On-device dataflow (per core; everything bf16 except fp32 psum/norms):
  S0+S1  x -> rms (ACT-engine) -> hnT per token half; QK matmuls for a half
      start as soon as its hnT is ready; RoPE in T-layout; V token-major
  S2  attention per head-pair, software-pipelined transposed-softmax (the
      30*tanh soft cap bounds logits so exp(l-30) needs no max pass); causal
      via block skip + 0/1 mask tiles; attnT bf16
  S3  wo row-shard matmul per token half -> RS1 chunk over tokens; rank m
      receives the interleaved slice {m*128..} u {512+m*128..}
  S4  rank chunk (128 tok): h1 = x + rms(ha)*s_post_attn (ACT/gpsimd-heavy,
      emitted at vector-idle points); hn2 bf16 -> AG chunk
  S4b gathered hn2 chunk -> PE-transpose to hn2T [D, 512]
  S5  per token group: gateT/valT matmuls, gelu_tanh * val -> actT bf16;
      weights re-streamed per group so AG chunk 1 hides under group 0
  S6  w_out row-shard matmul -> RS2 in 8 (D-quarter x token-half) chunks
  S7  rank chunks: out = h1 + rms(h_dense)*s_post_mlp; host reassembles the
      interleaved token slices

A dummy warmup AllReduce at kernel start absorbs the ~80us first-collective
ring-sync cost; all collectives are ordered so their flights hide under
attention/S3/S5 compute.
"""

import os

import numpy as np
import ml_dtypes

import concourse.bass as bass
import concourse.mybir as mybir
import concourse.tile as tile
from concourse import bacc
from concourse.bass_utils import run_bass_kernel_spmd
from concourse.masks import make_identity

F32 = mybir.dt.float32
F32R = mybir.dt.float32r
BF16 = mybir.dt.bfloat16
AF = mybir.ActivationFunctionType
ALU = mybir.AluOpType

# Problem dims
B, S, D = 2, 1024, 4096
NQ, NKV, HD = 32, 8, 128
FFN = 10928
ATTN_MULT = 0.08838834764831845
MAX_ATTN = 30.0
EPS = 1e-5
BASE = 10000.0

# Distribution
N_CORES = 8
TP = 4
RG = [[0, 1, 2, 3], [4, 5, 6, 7]]

# Per-core dims
T = S                      # tokens per core (its whole batch)
TC = T // 128              # 8 token chunks
TG = T // 512              # 2 token groups (matmul moving dim)
DC = D // 128              # 32 contraction chunks
DB = D // 512              # 8 output blocks
DH = D // 2
HQ = NQ // TP              # 8 q heads per core
HKV = NKV // TP            # 2 kv heads per core
GQ = NQ // NKV             # 4 q heads per kv head
FFN_PAD = 11264            # 22*128*4
F = FFN_PAD // TP          # 2816 ffn columns per core
FC = F // 128              # 22 ffn chunks
KC = S // 128              # 8 kv chunks
TSL = T // TP              # 256 tokens per rank after ReduceScatter
TSC = TSL // 128           # 2 chunks


def _dma_split(nc, dst, src, n=4):
    """Issue n parallel dma_starts covering dst/src sliced on their 2nd axis."""
    dims = dst.shape
    ax = 1
    size = dims[ax]
    step = max(1, size // n)
    i = 0
    while i < size:
        j = min(size, i + step)
        if len(dims) == 2:
            nc.sync.dma_start(dst[:, i:j], src[:, i:j])
        else:
            nc.sync.dma_start(dst[:, i:j, :], src[:, i:j, :])
        i = j


def _analyze_mask(mask_qk: np.ndarray):
    """Classify each (q-group-of-512, kv-chunk-of-128) block of mask[q, kv]:
    'skip' (all masked), 'full' (all visible), or a [128, 512] 0/1 tile in
    scoresT layout [kv, q]."""
    blocks = {}
    tiles = []
    for g in range(TG):
        for k in range(KC):
            blk = mask_qk[g * 512:(g + 1) * 512, k * 128:(k + 1) * 128]
            if not blk.any():
                blocks[(g, k)] = ("skip", -1)
            elif blk.all():
                blocks[(g, k)] = ("full", -1)
            else:
                blocks[(g, k)] = ("partial", len(tiles))
                tiles.append(blk.T.astype(np.float32))  # [kv 128, q 512]
    if not tiles:
        tiles.append(np.zeros((128, 512), np.float32))
    return blocks, np.stack(tiles)


def _build_nc(blocks, n_mask_tiles, sim_no_cc=False):
    nc = bacc.Bacc("TRN2", target_bir_lowering=False, debug=False,
                   num_devices=N_CORES)

    # Per-core external inputs
    x_d = nc.dram_tensor("x", [T, D], F32, kind="ExternalInput")
    xsl_d = nc.dram_tensor("xsl", [TSL, D], F32, kind="ExternalInput")
    wq_d = nc.dram_tensor("wq", [HQ, 128, DC, 128], BF16, kind="ExternalInput")
    wk_d = nc.dram_tensor("wk", [HKV, 128, DC, 128], BF16, kind="ExternalInput")
    wv_d = nc.dram_tensor("wv", [128, DC, HKV * HD], BF16, kind="ExternalInput")
    wo_d = nc.dram_tensor("wo", [DB // 2, 128, HQ, 1024], BF16,
                          kind="ExternalInput")
    wg_d = nc.dram_tensor("wg", [FC, 128, DC, 128], BF16, kind="ExternalInput")
    wv2_d = nc.dram_tensor("wv2", [FC, 128, DC, 128], BF16, kind="ExternalInput")
    wout_d = nc.dram_tensor("wout", [DB // 2, 128, FC, 1024], BF16,
                            kind="ExternalInput")
    spa_d = nc.dram_tensor("sp_attn", [128, D], F32, kind="ExternalInput")
    spm_d = nc.dram_tensor("sp_mlp", [128, D], F32, kind="ExternalInput")
    cosT_d = nc.dram_tensor("cosT", [HD, T], F32, kind="ExternalInput")
    sinTn_d = nc.dram_tensor("sinTn", [HD, T], F32, kind="ExternalInput")
    dmask_d = nc.dram_tensor("dmask", [n_mask_tiles, 128, 512], BF16,
                             kind="ExternalInput")
    out_d = nc.dram_tensor("out", [TSL, D], F32, kind="ExternalOutput")

    with tile.TileContext(nc) as tc:
        with (
            tc.tile_pool(name="dram", bufs=1, space="DRAM") as dram,
            tc.tile_pool(name="const", bufs=1) as const,
            tc.tile_pool(name="ms", bufs=4) as msp,
        ):
            # DRAM scratch
            qt_dram = dram.tile([HQ, HD, T], F32R)
            kt_dram = dram.tile([HKV, HD, T], F32R)
            v_dram = dram.tile([T, HKV * HD], BF16)
            DQ = D // 4
            # RS1/AG are chunked over token halves; rank m's slice is the
            # interleaved set {m*128..} u {512+m*128..} (chunk c = half c).
            rs1_in = [dram.tile([T // 2, D], BF16, name=f"rs1i{h}")
                      for h in range(2)]
            rs1_out = [dram.tile([128, D], BF16, name=f"rs1o{h}")
                       for h in range(2)]
            ag_in = [dram.tile([128, D], BF16, name=f"agi{c}")
                     for c in range(2)]
            ag_out = [dram.tile([T // 2, D], BF16, name=f"ago{c}")
                      for c in range(2)]
            # RS2 is chunked over (D quarter x token half): chunk p*2+th
            # covers D cols p*1024.. for tokens th*512..; rank m receives
            # its interleaved 128-token rows, matching RS1's mapping.
            rs2_in = [dram.tile([T // 2, DQ], BF16, name=f"rs2i{h}")
                      for h in range(8)]
            rs2_out = [dram.tile([128, DQ], BF16, name=f"rs2o{h}")
                       for h in range(8)]
            h1_dram = dram.tile([TSL, D], F32)

            def _rs(in_t, out_t):
                if sim_no_cc:
                    n = out_t.shape[0]
                    nc.sync.dma_start(out_t[:], in_t[0:n, :])
                else:
                    nc.gpsimd.collective_compute(
                        "ReduceScatter", ALU.add, replica_groups=RG,
                        ins=[in_t[:].opt()], outs=[out_t[:].opt()])

            def _ag(in_t, out_t):
                if sim_no_cc:
                    n = in_t.shape[0]
                    for r in range(TP):
                        nc.sync.dma_start(out_t[r * n:(r + 1) * n, :], in_t[:])
                else:
                    nc.gpsimd.collective_compute(
                        "AllGather", ALU.bypass, replica_groups=RG,
                        ins=[in_t[:].opt()], outs=[out_t[:].opt()])

            ident = const.tile([128, 128], F32)
            make_identity(nc, ident)
            ident_bf = const.tile([128, 128], BF16)
            nc.vector.tensor_copy(ident_bf[:], ident[:])
            ones_col_f = const.tile([128, 1], F32)
            nc.vector.memset(ones_col_f[:], 1.0)
            ones_col = const.tile([128, 1], BF16)
            nc.vector.tensor_copy(ones_col[:], ones_col_f[:])
            ones_row_f = const.tile([1, 128], F32)
            nc.vector.memset(ones_row_f[:], 1.0)
            ones_row = const.tile([1, 128], BF16)
            nc.vector.tensor_copy(ones_row[:], ones_row_f[:])
            eps_col = const.tile([128, 1], F32)
            nc.vector.memset(eps_col[:], EPS)
            negcap_col = const.tile([128, 1], F32)
            nc.vector.memset(negcap_col[:], -MAX_ATTN)

            # Tiny warmup AllReduce: absorbs the first-collective ring/sync
            # setup cost (~80us) under S0/S1 compute so RS1 runs at data speed
            warm_in = dram.tile([1, 512], F32, name="warm_i")
            warm_out = dram.tile([1, 512], F32, name="warm_o")
            wz = const.tile([1, 512], F32)
            nc.vector.memset(wz[:], 0.0)
            nc.sync.dma_start(warm_in[:], wz[:])
            if not sim_no_cc:
                nc.gpsimd.collective_compute(
                    "AllReduce", ALU.add, replica_groups=RG,
                    ins=[warm_in[:].opt()], outs=[warm_out[:].opt()])

            # ---------------- S0+S1 interleaved: rms/transpose per token
            # half, QK matmuls for a half start as soon as its hnT is ready
            # (weights re-streamed per half; V at the end needs full hnT) ----
            hnT_cm = tc.tile_pool(name="hnT_pool", bufs=1)
            hnT_pool = hnT_cm.__enter__()
            hnT = hnT_pool.tile([128, DC, T], BF16, name="hnT")
            with (
                tc.tile_pool(name="s0", bufs=2) as s0,
                tc.tile_pool(name="s0b", bufs=2) as s0b,
                tc.tile_pool(name="s0scr", bufs=1) as s0scr,
                tc.tile_pool(name="ps0", bufs=4, space="PSUM") as ps0,
                tc.tile_pool(name="s1t", bufs=3) as s1t,
                tc.tile_pool(name="s1c", bufs=1) as s1c,
                tc.tile_pool(name="s1w", bufs=3) as s1w,
                tc.tile_pool(name="ps1", bufs=2, space="PSUM") as ps1,
                nc.named_scope("s01_rms_qkv"),
            ):
                cosT_sb = s1c.tile([HD, T], F32)
                nc.sync.dma_start(cosT_sb[:], cosT_d[:])
                sinTn_sb = s1c.tile([HD, T], F32)
                nc.sync.dma_start(sinTn_sb[:], sinTn_d[:])

                def s0_chunk(t):
                    x_t = s0.tile([128, D], F32, tag="x_t")
                    _dma_split(nc, x_t[:], x_d[t * 128:(t + 1) * 128, :], 4)
                    scr_t = s0scr.tile([128, D], F32, tag="scr_t")
                    ms_t = msp.tile([128, 1], F32, tag="ms")
                    # rms on ACT + gpsimd: vector only drains psums
                    nc.scalar.activation(scr_t[:], x_t[:], AF.Square,
                                         accum_out=ms_t[:])
                    sq_t = msp.tile([128, 1], F32, tag="sq0")
                    nc.scalar.activation(sq_t[:], ms_t[:], AF.Sqrt,
                                         bias=eps_col[:], scale=1.0 / D)
                    inv_t = msp.tile([128, 1], F32, tag="inv")
                    nc.gpsimd.normalize_recip(inv_t[:], ones_col_f[:], sq_t[:])
                    hn_t = s0b.tile([128, D], BF16, tag="hn_t")
                    nc.scalar.activation(hn_t[:], x_t[:], AF.Copy,
                                         scale=inv_t[:])
                    for dc in range(DC):
                        pt = ps0.tile([128, 128], BF16, tag="pt")
                        nc.tensor.transpose(
                            pt[:], hn_t[:, dc * 128:(dc + 1) * 128],
                            ident_bf[:])
                        nc.vector.tensor_copy(
                            hnT[:, dc, t * 128:(t + 1) * 128], pt[:])

                def rope_store(psum, dst, g):
                    """psum [128 d, 512 tok] -> rope -> DMA to dst [128, 512]."""
                    cs = cosT_sb[:, g * 512:(g + 1) * 512]
                    sn = sinTn_sb[:, g * 512:(g + 1) * 512]
                    raw = s1t.tile([128, 512], F32, tag="rp_raw")
                    nc.vector.tensor_copy(raw[:], psum[:])
                    rot = s1t.tile([128, 512], F32, tag="rp_rot")
                    nc.sync.dma_start(rot[0:64, :], raw[64:128, :])
                    nc.sync.dma_start(rot[64:128, :], raw[0:64, :])
                    oz = s1t.tile([128, 512], F32R, tag="rp_out")
                    nc.vector.tensor_tensor(oz[:], raw[:], cs, ALU.mult)
                    nc.vector.tensor_tensor(rot[:], rot[:], sn, ALU.mult)
                    nc.vector.tensor_tensor(oz[:], oz[:], rot[:], ALU.add)
                    nc.sync.dma_start(dst, oz[:])

                def s1_qk_pass(g):
                    for cb in range(HQ + HKV):  # q heads then kv heads
                        is_q = cb < HQ
                        w_cb = s1w.tile([128, DC, 128], BF16, tag="w_cb")
                        _dma_split(nc, w_cb[:],
                                   wq_d[cb] if is_q else wk_d[cb - HQ], 8)
                        pq = ps1.tile([128, 512], F32, tag="pq")
                        for dc in range(DC):
                            nc.tensor.matmul(
                                pq[:], w_cb[:, dc, :],
                                hnT[:, dc, g * 512:(g + 1) * 512],
                                start=(dc == 0), stop=(dc == DC - 1))
                        dst = (qt_dram[cb, :, g * 512:(g + 1) * 512] if is_q
                               else kt_dram[cb - HQ, :, g * 512:(g + 1) * 512])
                        rope_store(pq, dst, g)

                for t in range(4):
                    s0_chunk(t)
                s1_qk_pass(0)
                for t in range(4, TC):
                    s0_chunk(t)
                s1_qk_pass(1)

                # V token-major
                wv_sb = s1c.tile([128, DC, HKV * HD], BF16)
                _dma_split(nc, wv_sb[:], wv_d[:], 8)
                for t in range(TC):
                    pv = ps1.tile([128, 256], F32, tag="pv")
                    for dc in range(DC):
                        nc.tensor.matmul(
                            pv[:], hnT[:, dc, t * 128:(t + 1) * 128],
                            wv_sb[:, dc, :],
                            start=(dc == 0), stop=(dc == DC - 1))
                    vz = s1t.tile([128, 256], BF16, tag="vz")
                    nc.vector.tensor_copy(vz[:], pv[:])
                    nc.sync.dma_start(v_dram[t * 128:(t + 1) * 128, :], vz[:])

            # ---- S2+S3+S4 fused: attention (g-major) -> wo -> RS1 per token
            # half -> post-attn norm (gpsimd) -> AG per rank chunk. Emission
            # is scheduled so every collective flight hides under compute. ----
            hnT_cm.__exit__(None, None, None)
            s4p_cm = tc.tile_pool(name="s4p", bufs=1)
            s4p = s4p_cm.__enter__()
            s4b_cm = tc.tile_pool(name="s4b", bufs=1)
            s4b = s4b_cm.__enter__()
            attnT_cm = tc.tile_pool(name="attnT_pool", bufs=1)
            attnT_pool = attnT_cm.__enter__()
            attnT = attnT_pool.tile([128, HQ, T], BF16, name="attnT")
            with (
                tc.tile_pool(name="s2c", bufs=1) as s2c,
                tc.tile_pool(name="s2t", bufs=2) as s2t,
                tc.tile_pool(name="s2sc", bufs=1) as s2sc,
                tc.tile_pool(name="s3w", bufs=2) as s3w,
                tc.tile_pool(name="s3t", bufs=3) as s3t,
                tc.tile_pool(name="ps2s", bufs=2, space="PSUM") as ps2s,
                tc.tile_pool(name="ps2x", bufs=1, space="PSUM") as ps2x,
                tc.tile_pool(name="ps2o", bufs=1, space="PSUM") as ps2o,
                tc.tile_pool(name="ps2m", bufs=1, space="PSUM") as ps2m,
                tc.tile_pool(name="ps3", bufs=1, space="PSUM") as ps3,
                nc.named_scope("s234"),
            ):
                kt_sb = s2c.tile([128, HKV, T], F32R)
                v_sb = s2c.tile([128, KC, HKV * HD], BF16)
                _dma_split(nc, kt_sb[:], kt_dram[:].rearrange("h p t -> p h t"), 2)
                _dma_split(nc, v_sb[:],
                           v_dram[:].rearrange("(kc p) c -> p kc c", p=128), 4)
                dmask_sb = s2c.tile([128, n_mask_tiles, 512], BF16)
                nc.sync.dma_start(
                    dmask_sb[:], dmask_d[:].rearrange("n p q -> p n q"))
                spa_sb = s4b.tile([128, D], F32)
                nc.sync.dma_start(spa_sb[:], spa_d[:])

                def s2_scores(hp, g):
                    """Scores + tanh for head pair hp, token group g."""
                    ks = [k for k in range(KC) if blocks[(g, k)][0] != "skip"]
                    scs = []
                    for j in range(2):
                        h = 2 * hp + j
                        kv = h // GQ
                        qt_sb = s2t.tile([128, 512], F32R, tag=f"qt{j}")
                        nc.sync.dma_start(
                            qt_sb[:], qt_dram[h, :, g * 512:(g + 1) * 512])
                        tiles = []
                        for i, k in enumerate(ks):
                            psc = ps2s.tile([128, 512], F32, tag="psc")
                            nc.tensor.matmul(
                                psc[:],
                                kt_sb[:, kv, k * 128:(k + 1) * 128],
                                qt_sb[:], start=True, stop=True)
                            sc_t = s2sc.tile([128, 512], F32, tag=f"sc{j}_{i}")
                            nc.scalar.activation(sc_t[:], psc[:], AF.Tanh,
                                                 scale=ATTN_MULT / MAX_ATTN)
                            tiles.append(sc_t)
                        scs.append(tiles)
                    return ks, scs

                def s2_pv(hp, g, ks, scs):
                    """exp + mask + PV/sum accumulation + normalize for pair."""
                    psums = ps2m.tile([33, 512], F32, tag="psums")
                    for j in range(2):
                        h = 2 * hp + j
                        kv = h // GQ
                        po = ps2o.tile([128, 512], F32, tag=f"po{j}")
                        for i, k in enumerate(ks):
                            kind, mi = blocks[(g, k)]
                            p_t = s2t.tile([128, 512], BF16, tag=f"pt{j}")
                            nc.scalar.activation(p_t[:], scs[j][i][:],
                                                 AF.Exp, scale=MAX_ATTN,
                                                 bias=negcap_col[:])
                            if kind == "partial":
                                nc.vector.tensor_tensor(
                                    p_t[:], p_t[:], dmask_sb[:, mi, :],
                                    ALU.mult)
                            first = i == 0
                            last = i == len(ks) - 1
                            nc.tensor.matmul(
                                po[:],
                                v_sb[:, k, kv * 128:(kv + 1) * 128],
                                p_t[:], start=first, stop=last)
                            nc.tensor.matmul(
                                psums[32 * j:32 * j + 1, :], ones_col[:],
                                p_t[:], start=first, stop=last)
                        recip = s2t.tile([1, 512], BF16, tag=f"recip{j}")
                        with nc.allow_low_precision(
                                reason="softmax 1/sum broadcast via bf16 PE"):
                            nc.vector.reciprocal(
                                recip[:], psums[32 * j:32 * j + 1, :])
                        pbc = ps2x.tile([128, 512], F32, tag="pbc")
                        nc.tensor.matmul(pbc[:], ones_row[:], recip[:],
                                         start=True, stop=True)
                        rb = s2t.tile([128, 512], F32, tag=f"rb{j}")
                        nc.vector.tensor_copy(rb[:], pbc[:])
                        nc.vector.tensor_tensor(
                            attnT[:, h, g * 512:(g + 1) * 512], po[:], rb[:],
                            ALU.mult)

                def s3_p(th, p):
                    """wo matmul, one weight db-pair for token half th."""
                    wo_p = s3w.tile([128, HQ, 1024], BF16, tag="wo_p")
                    _dma_split(nc, wo_p[:], wo_d[p], 4)
                    for tt in range(4):
                        t = th * 4 + tt
                        pwA = ps3.tile([128, 512], F32, tag="pwA")
                        pwB = ps3.tile([128, 512], F32, tag="pwB")
                        for h in range(HQ):
                            at = attnT[:, h, t * 128:(t + 1) * 128]
                            nc.tensor.matmul(
                                pwA[:], at, wo_p[:, h, 0:512],
                                start=(h == 0), stop=(h == HQ - 1))
                            nc.tensor.matmul(
                                pwB[:], at, wo_p[:, h, 512:1024],
                                start=(h == 0), stop=(h == HQ - 1))
                        oz = s3t.tile([128, 1024], BF16, tag="oz")
                        nc.vector.tensor_copy(oz[:, 0:512], pwA[:])
                        nc.vector.tensor_copy(oz[:, 512:1024], pwB[:])
                        nc.sync.dma_start(
                            rs1_in[th][tt * 128:(tt + 1) * 128,
                                       p * 1024:(p + 1) * 1024], oz[:])

                def s4_early(c):
                    """Kick off the rank-chunk loads: ha on the gpsimd ring
                    (it must wait for RS1-c), x on sync (no deps)."""
                    ha = s4p.tile([128, D], BF16, tag="ha")
                    for i in range(4):
                        nc.gpsimd.dma_start(
                            ha[:, i * 1024:(i + 1) * 1024],
                            rs1_out[c][:, i * 1024:(i + 1) * 1024])
                    x_t = s4p.tile([128, D], F32, tag="x_t")
                    _dma_split(nc, x_t[:], xsl_d[c * 128:(c + 1) * 128, :], 4)
                    return ha, x_t

                def s4_late(c, ha, x_t):
                    """h1 + hn2, mostly on ACT/gpsimd (only 2 vector ops) so
                    the chain never blocks a busy vector queue -> AG."""
                    h1_t = s4b.tile([128, D], F32, tag="h1_t")
                    ms_t = msp.tile([128, 1], F32, tag="ms4")
                    # h1_t doubles as the ha^2 scratch before being overwritten
                    nc.scalar.activation(h1_t[:], ha[:], AF.Square,
                                         accum_out=ms_t[:])
                    sq_t = msp.tile([128, 1], F32, tag="sq4")
                    nc.scalar.activation(sq_t[:], ms_t[:], AF.Sqrt,
                                         bias=eps_col[:], scale=1.0 / D)
                    inv_t = msp.tile([128, 1], F32, tag="inv4")
                    nc.gpsimd.normalize_recip(inv_t[:], ones_col_f[:], sq_t[:])
                    nc.vector.scalar_tensor_tensor(
                        h1_t[:], ha[:], inv_t[:], spa_sb[:],
                        op0=ALU.mult, op1=ALU.mult)
                    nc.vector.tensor_tensor(h1_t[:], h1_t[:], x_t[:], ALU.add)
                    for i in range(4):
                        nc.gpsimd.dma_start(
                            h1_dram[c * 128:(c + 1) * 128,
                                    i * 1024:(i + 1) * 1024],
                            h1_t[:, i * 1024:(i + 1) * 1024])
                    ms2_t = msp.tile([128, 1], F32, tag="ms4b")
                    # x_t consumed by the residual add; reuse as h1^2 scratch
                    nc.scalar.activation(x_t[:], h1_t[:], AF.Square,
                                         accum_out=ms2_t[:])
                    sq2_t = msp.tile([128, 1], F32, tag="sq4b")
                    nc.scalar.activation(sq2_t[:], ms2_t[:], AF.Sqrt,
                                         bias=eps_col[:], scale=1.0 / D)
                    inv2_t = msp.tile([128, 1], F32, tag="inv4b")
                    nc.gpsimd.normalize_recip(inv2_t[:], ones_col_f[:],
                                              sq2_t[:])
                    hn2b_t = s4b.tile([128, D], BF16, tag="hn2b")
                    nc.scalar.activation(hn2b_t[:], h1_t[:], AF.Copy,
                                         scale=inv2_t[:])
                    for i in range(4):
                        nc.gpsimd.dma_start(
                            ag_in[c][:, i * 1024:(i + 1) * 1024],
                            hn2b_t[:, i * 1024:(i + 1) * 1024])
                    _ag(ag_in[c], ag_out[c])

                # Emission schedule: S3 half-0's weight-pairs interleave into
                # g1's attention stream (filling PE gaps of the ACT-bound
                # softmax); RS1-t0 fires as attention drains; AG-c0 during S3
                # half 1; RS1-t1 + AG-c1 hide under S5's first pass.
                pairs0 = [(hp, 0) for hp in range(HQ // 2)]
                pairs1 = [(hp, 1) for hp in range(HQ // 2)]
                prev = None
                for cur in pairs0:
                    info = s2_scores(*cur)
                    if prev is not None:
                        s2_pv(*prev)
                    prev = (cur[0], cur[1], info[0], info[1])
                s2_pv(*prev)
                i10 = s2_scores(*pairs1[0])
                for p in range(DB // 2):
                    s3_p(0, p)
                _rs(rs1_in[0], rs1_out[0])
                st0 = s4_early(0)
                prev = (pairs1[0][0], 1, i10[0], i10[1])
                for cur in pairs1[1:]:
                    info = s2_scores(*cur)
                    s2_pv(*prev)
                    prev = (cur[0], cur[1], info[0], info[1])
                s2_pv(*prev)
                s3_p(1, 0)
                s4_late(0, *st0)
                for p in range(1, DB // 2):
                    s3_p(1, p)
                _rs(rs1_in[1], rs1_out[1])
                st1 = s4_early(1)
            attnT_cm.__exit__(None, None, None)

            # ------- S4b+S5 per token group: transpose gathered hn2 chunk
            # into a half-T buffer, then that group's MLP pass (weights
            # re-streamed per pass; chunk 1's norm + AG are emitted after
            # group 0's transposes so the flight hides under S5 pass 0) ------
            actT_cm = tc.tile_pool(name="actT_pool", bufs=1)
            actT_pool = actT_cm.__enter__()
            actT = actT_pool.tile([128, FC, T], BF16, name="actT")
            with (
                tc.tile_pool(name="s4t", bufs=2) as s4t,
                tc.tile_pool(name="hn2Tp", bufs=1) as hn2Tp,
                tc.tile_pool(name="ps4", bufs=4, space="PSUM") as ps4,
                tc.tile_pool(name="s5w", bufs=2) as s5w,
                tc.tile_pool(name="s5t", bufs=2) as s5t,
                tc.tile_pool(name="ps5", bufs=2, space="PSUM") as ps5,
                nc.named_scope("s45_mlp"),
            ):
                for g in range(TG):
                    # ag_out[g] holds global tokens g*512..(g+1)*512 in order
                    hn2T = hn2Tp.tile([128, DC, 512], BF16, tag="hn2Tg")
                    for r in range(TP):
                        hb = s4t.tile([128, D], BF16, tag="hb")
                        _dma_split(nc, hb[:],
                                   ag_out[g][r * 128:(r + 1) * 128, :], 4)
                        for dc in range(DC):
                            ptb = ps4.tile([128, 128], BF16, tag="ptb")
                            nc.tensor.transpose(
                                ptb[:], hb[:, dc * 128:(dc + 1) * 128],
                                ident_bf[:])
                            nc.vector.tensor_copy(
                                hn2T[:, dc, r * 128:(r + 1) * 128], ptb[:])
                    for f in range(FC):
                        if g == 0 and f == 4:
                            # mid-pass: ha1 has landed by the time vector
                            # reaches this, and AG-c1 still hides under the
                            # rest of pass 0
                            s4_late(1, *st1)
                        wg_f = s5w.tile([128, DC, 128], BF16, tag="wg_f")
                        _dma_split(nc, wg_f[:], wg_d[f], 8)
                        wv2_f = s5w.tile([128, DC, 128], BF16, tag="wv2_f")
                        _dma_split(nc, wv2_f[:], wv2_d[f], 8)
                        pg = ps5.tile([128, 512], F32, tag="pg")
                        pv2 = ps5.tile([128, 512], F32, tag="pv")
                        for dc in range(DC):
                            nc.tensor.matmul(
                                pg[:], wg_f[:, dc, :], hn2T[:, dc, :],
                                start=(dc == 0), stop=(dc == DC - 1))
                        for dc in range(DC):
                            nc.tensor.matmul(
                                pv2[:], wv2_f[:, dc, :], hn2T[:, dc, :],
                                start=(dc == 0), stop=(dc == DC - 1))
                        gel = s5t.tile([128, 512], F32, tag="gel")
                        nc.scalar.activation(gel[:], pg[:], AF.Gelu_apprx_tanh)
                        nc.vector.tensor_tensor(
                            actT[:, f, g * 512:(g + 1) * 512], gel[:], pv2[:],
                            ALU.mult)

            # ------- S6: w_out (row shard, db-pair blocked) -> RS2 -----------
            with (
                tc.tile_pool(name="s6w", bufs=2) as s6w,
                tc.tile_pool(name="s6t", bufs=3) as s6t,
                tc.tile_pool(name="ps6", bufs=2, space="PSUM") as ps6,
                nc.named_scope("s6_wout"),
            ):
                for p in range(DB // 2):
                    wout_p = s6w.tile([128, FC, 1024], BF16, tag="wout_p")
                    _dma_split(nc, wout_p[:], wout_d[p], 8)
                    for t in range(TC):
                        pdA = ps6.tile([128, 512], F32, tag="pdA")
                        pdB = ps6.tile([128, 512], F32, tag="pdB")
                        for f in range(FC):
                            a = actT[:, f, t * 128:(t + 1) * 128]
                            nc.tensor.matmul(
                                pdA[:], a, wout_p[:, f, 0:512],
                                start=(f == 0), stop=(f == FC - 1))
                            nc.tensor.matmul(
                                pdB[:], a, wout_p[:, f, 512:1024],
                                start=(f == 0), stop=(f == FC - 1))
                        oz = s6t.tile([128, 1024], BF16, tag="oz6")
                        nc.vector.tensor_copy(oz[:, 0:512], pdA[:])
                        nc.vector.tensor_copy(oz[:, 512:1024], pdB[:])
                        cix = p * 2 + t // 4
                        nc.sync.dma_start(
                            rs2_in[cix][(t % 4) * 128:(t % 4 + 1) * 128, :],
                            oz[:])
                        if t % 4 == 3:
                            _rs(rs2_in[cix], rs2_out[cix])

            actT_cm.__exit__(None, None, None)
            s4b_cm.__exit__(None, None, None)
            s4p_cm.__exit__(None, None, None)
            # ------- S7: rank slice: out = h1 + rms(h_dense)*s_post_mlp ------
            with (
                tc.tile_pool(name="s7", bufs=2) as s7,
                tc.tile_pool(name="s7b", bufs=1) as s7b,
                nc.named_scope("s7_out"),
            ):
                spm_sb = s7b.tile([128, D], F32)
                nc.sync.dma_start(spm_sb[:], spm_d[:])
                DQ4 = D // 4
                for t in range(TSC):
                    h1_t = s7.tile([128, D], F32, tag="h1r")
                    _dma_split(nc, h1_t[:], h1_dram[t * 128:(t + 1) * 128, :], 4)
                    scr = s7.tile([128, D], F32, tag="scr7")
                    ms_t = msp.tile([128, 1], F32, tag="ms7")
                    hdq = []
                    # squares on ACT so each quarter reduces as its RS2 chunk
                    # lands, without waiting on the vector queue
                    for q in range(4):
                        hd = s7.tile([128, DQ4], BF16, tag=f"hd{q}")
                        _dma_split(nc, hd[:], rs2_out[q * 2 + t][:], 2)
                        hdq.append(hd)
                        if q == 0:
                            nc.scalar.activation(scr[:, 0:DQ4], hd[:],
                                                 AF.Square,
                                                 accum_out=ms_t[:])
                        else:
                            msq = msp.tile([128, 1], F32, tag=f"ms7q{q}")
                            nc.scalar.activation(
                                scr[:, q * DQ4:(q + 1) * DQ4], hd[:],
                                AF.Square, accum_out=msq[:])
                            nc.vector.tensor_tensor(ms_t[:], ms_t[:], msq[:],
                                                    ALU.add)
                    inv_t = msp.tile([128, 1], F32, tag="inv7")
                    nc.scalar.activation(inv_t[:], ms_t[:], AF.Sqrt,
                                         bias=eps_col[:], scale=1.0 / D)
                    nc.vector.reciprocal(inv_t[:], inv_t[:])
                    o_t = s7.tile([128, D], F32, tag="o_t")
                    for q in range(4):
                        nc.vector.scalar_tensor_tensor(
                            o_t[:, q * DQ4:(q + 1) * DQ4], hdq[q][:],
                            inv_t[:], spm_sb[:, q * DQ4:(q + 1) * DQ4],
                            op0=ALU.mult, op1=ALU.mult)
                    nc.vector.tensor_tensor(o_t[:], o_t[:], h1_t[:], ALU.add)
                    _dma_split(nc, out_d[t * 128:(t + 1) * 128, :], o_t[:], 4)

    nc.compile()
    return nc


_NC_CACHE = {}
_FN_CACHE = {}
LAST_RESULTS = None


def _get_sharded_fn(nc):
    """Build (once) the jitted shard_map callable for `nc` across 8 cores.

    Mirrors concourse.bass2jax.run_bass_via_pjrt's multi-core path, but caches
    the compiled function and takes pre-sharded device arrays so repeated calls
    can be timed without re-shipping inputs.
    """
    if id(nc) in _FN_CACHE:
        return _FN_CACHE[id(nc)]
    import jax
    from jax.sharding import Mesh, PartitionSpec
    from jax.experimental.shard_map import shard_map
    from concourse import bass2jax as b2j

    b2j.install_neuronx_cc_hook()
    part_name = nc.partition_id_tensor.name if nc.partition_id_tensor else None
    in_names, out_names, out_avals, zero_outs = [], [], [], []
    for alloc in nc.m.functions[0].allocations:
        if not isinstance(alloc, mybir.MemoryLocationSet):
            continue
        name = alloc.memorylocations[0].name
        if alloc.kind == "ExternalInput":
            if name == part_name:
                continue
            in_names.append(name)
        elif alloc.kind == "ExternalOutput":
            out_names.append(name)
            shape = tuple(alloc.tensor_shape)
            dtype = mybir.dt.np(alloc.dtype)
            out_avals.append(jax.core.ShapedArray(shape, dtype))
            zero_outs.append(np.zeros(shape, dtype))
    n_params = len(in_names)
    all_names = in_names + out_names
    if part_name is not None:
        all_names = all_names + [part_name]

    def _body(*args):
        operands = list(args)
        if part_name is not None:
            operands.append(b2j.partition_id_tensor())
        outs = b2j._bass_exec_p.bind(
            *operands,
            out_avals=tuple(out_avals),
            in_names=tuple(all_names),
            out_names=tuple(out_names),
            lowering_input_output_aliases=(),
            sim_require_finite=True,
            sim_require_nnan=True,
            nc=nc,
        )
        return tuple(outs)

    devices = jax.devices()[:N_CORES]
    mesh = Mesh(np.asarray(devices), ("core",))
    n_outs = len(out_names)
    donate = tuple(range(n_params, n_params + n_outs))
    sharded = jax.jit(
        shard_map(
            _body,
            mesh=mesh,
            in_specs=(PartitionSpec("core"),) * (n_params + n_outs),
            out_specs=(PartitionSpec("core"),) * n_outs,
            check_rep=False,
        ),
        donate_argnums=donate,
        keep_unused=True,
    )
    entry = dict(
        fn=sharded, in_names=in_names, out_names=out_names,
        out_avals=out_avals, zero_outs=zero_outs, mesh=mesh,
    )
    _FN_CACHE[id(nc)] = entry
    return entry


def _device_inputs(nc, in_maps):
    import jax
    from jax.sharding import NamedSharding, PartitionSpec

    entry = _get_sharded_fn(nc)
    sh = NamedSharding(entry["mesh"], PartitionSpec("core"))
    concat_in = [
        np.concatenate([np.asarray(m[name]) for m in in_maps], axis=0)
        for name in entry["in_names"]
    ]
    return [jax.device_put(a, sh) for a in concat_in]


def _dev_zeros(nc):
    import jax
    from jax.sharding import NamedSharding, PartitionSpec

    entry = _get_sharded_fn(nc)
    sh = NamedSharding(entry["mesh"], PartitionSpec("core"))
    return [
        jax.device_put(
            np.zeros((N_CORES * z.shape[0], *z.shape[1:]), z.dtype), sh)
        for z in entry["zero_outs"]
    ]


def _run(nc, dev_in):
    entry = _get_sharded_fn(nc)
    out_arrs = entry["fn"](*dev_in, *_dev_zeros(nc))
    outs = []
    for i, name in enumerate(entry["out_names"]):
        shp = entry["out_avals"][i].shape
        outs.append(np.asarray(out_arrs[i]).reshape(N_CORES, *shp))
    return dict(zip(entry["out_names"], outs))


def _run_timed(nc, dev_in, iters=5):
    """Returns (per-call wall seconds list). Inputs already device-resident;
    donated zero buffers are re-staged outside the timed window."""
    import time as _time

    entry = _get_sharded_fn(nc)
    times = []
    for _ in range(iters):
        zeros = _dev_zeros(nc)
        for z in zeros:
            z.block_until_ready()
        t0 = _time.perf_counter()
        out = entry["fn"](*dev_in, *zeros)
        for o in out:
            o.block_until_ready()
        times.append(_time.perf_counter() - t0)
    return times


def _arr_qk(w, nchunks):
    # [D, nchunks*128] -> [nchunks, 128(part), DC, 128]; fully contiguous DMA
    d, c = w.shape
    return np.ascontiguousarray(
        w.reshape(d // 128, 128, nchunks, 128).transpose(2, 1, 0, 3))


def _arr_v(w):
    # [D, C] -> [128(part), DC, C]
    d, c = w.shape
    return np.ascontiguousarray(w.reshape(d // 128, 128, c).transpose(1, 0, 2))


def _arr_o_pair(w, nchunks):
    # [nchunks*128, D] -> [D//1024, 128(part), nchunks, 1024] (db-pair blocks)
    r, d = w.shape
    return np.ascontiguousarray(
        w.reshape(nchunks, 128, d // 1024, 1024).transpose(2, 1, 0, 3))


def _prepare(inputs):
    x = np.asarray(inputs["x"], np.float32)
    mask_qk = np.asarray(inputs["mask"]).reshape(S, S).astype(bool)
    s_pre_attn = np.asarray(inputs["scale_pre_attn"], np.float32)
    s_post_attn = np.asarray(inputs["scale_post_attn"], np.float32)
    s_pre_mlp = np.asarray(inputs["scale_pre_mlp"], np.float32)
    s_post_mlp = np.asarray(inputs["scale_post_mlp"], np.float32)
    wq = np.asarray(inputs["wq"], np.float32) * s_pre_attn[:, None]
    wk = np.asarray(inputs["wk"], np.float32) * s_pre_attn[:, None]
    wv = np.asarray(inputs["wv"], np.float32) * s_pre_attn[:, None]
    wo = np.asarray(inputs["wo"], np.float32)
    wg = np.asarray(inputs["w_gate"], np.float32) * s_pre_mlp[:, None]
    wv2 = np.asarray(inputs["w_val"], np.float32) * s_pre_mlp[:, None]
    wout = np.asarray(inputs["w_out"], np.float32)

    blocks, dmask = _analyze_mask(mask_qk)
    key = tuple(sorted((k, v[0], v[1]) for k, v in blocks.items()))
    if key not in _NC_CACHE:
        _NC_CACHE[key] = _build_nc(blocks, dmask.shape[0])
    nc = _NC_CACHE[key]

    # FFN zero-padding to a multiple of 512 (22*128 per TP rank)
    wg_p = np.zeros((D, FFN_PAD), ml_dtypes.bfloat16)
    wg_p[:, :FFN] = wg.astype(ml_dtypes.bfloat16)
    wv2_p = np.zeros((D, FFN_PAD), ml_dtypes.bfloat16)
    wv2_p[:, :FFN] = wv2.astype(ml_dtypes.bfloat16)
    wout_p = np.zeros((FFN_PAD, D), ml_dtypes.bfloat16)
    wout_p[:FFN, :] = wout.astype(ml_dtypes.bfloat16)
    wo_bf = wo.astype(ml_dtypes.bfloat16)

    # RoPE tables in T-layout
    inv_freq = 1.0 / (BASE ** (np.arange(0, HD, 2, dtype=np.float64) / HD))
    phase = np.arange(S, dtype=np.float64)[:, None] * inv_freq[None, :]
    cos_f = np.cos(phase).astype(np.float32)   # [S, 64]
    sin_f = np.sin(phase).astype(np.float32)
    cosT = np.concatenate([cos_f.T, cos_f.T], axis=0)           # [128, S]
    sinTn = np.concatenate([-sin_f.T, sin_f.T], axis=0)         # [128, S]

    spa_bc = np.ascontiguousarray(
        np.broadcast_to(s_post_attn, (128, D)), dtype=np.float32)
    spm_bc = np.ascontiguousarray(
        np.broadcast_to(s_post_mlp, (128, D)), dtype=np.float32)

    in_maps = []
    for c in range(N_CORES):
        b, m = c // TP, c % TP
        in_maps.append({
            "x": np.ascontiguousarray(x[b]),
            "xsl": np.ascontiguousarray(np.concatenate([
                x[b][m * 128:(m + 1) * 128],
                x[b][512 + m * 128:512 + (m + 1) * 128]])),
            "wq": _arr_qk(wq[:, m * HQ * HD:(m + 1) * HQ * HD], HQ
                          ).astype(ml_dtypes.bfloat16),
            "wk": _arr_qk(wk[:, m * HKV * HD:(m + 1) * HKV * HD], HKV
                          ).astype(ml_dtypes.bfloat16),
            "wv": _arr_v(wv[:, m * HKV * HD:(m + 1) * HKV * HD]
                         ).astype(ml_dtypes.bfloat16),
            "wo": _arr_o_pair(wo_bf[m * HQ * HD:(m + 1) * HQ * HD, :], HQ),
            "wg": _arr_qk(wg_p[:, m * F:(m + 1) * F], FC),
            "wv2": _arr_qk(wv2_p[:, m * F:(m + 1) * F], FC),
            "wout": _arr_o_pair(wout_p[m * F:(m + 1) * F, :], FC),
            "sp_attn": spa_bc,
            "sp_mlp": spm_bc,
            "cosT": cosT,
            "sinTn": sinTn,
            "dmask": dmask.astype(ml_dtypes.bfloat16),
        })

    return nc, in_maps


def _assemble(out_percore):
    """out_percore: [N_CORES, TSL, D] -> [B, S, D]. Rank m holds the
    interleaved token slice {m*128..} u {512+m*128..} (chunk-major)."""
    out = np.empty((B, S, D), np.float32)
    for b in range(B):
        for m in range(TP):
            res = out_percore[b * TP + m]
            out[b, m * 128:(m + 1) * 128] = res[0:128]
            out[b, 512 + m * 128:512 + (m + 1) * 128] = res[128:256]
    return out


def kernel(**inputs):
    global LAST_RESULTS
    nc, in_maps = _prepare(inputs)
    from concourse._compat import axon_active
    if axon_active():
        # axon client: cached jit/shard_map path (run_bass_kernel_spmd would
        # also work but re-ships inputs per call)
        dev_in = _device_inputs(nc, in_maps)
        res = _run(nc, dev_in)
        LAST_RESULTS = res
        out = _assemble(res["out"])
    else:
        # native path (real /dev/neuron*): NRT execution, NTFF-capable
        r = run_bass_kernel_spmd(nc, in_maps, core_ids=list(range(N_CORES)))
        LAST_RESULTS = r
        out = _assemble(np.stack([r.results[c]["out"]
                                  for c in range(N_CORES)]))
    return out.astype(np.float32)
